# revision 1
# baseline (speedup 1.0000x reference)
"""Trainium2 Bass kernel for nn_DCTLayer: 8x8 block DCT-II followed by its exact
inverse (torch_dct norm=None convention). The DCT->IDCT round trip is the
identity map, so the layer reduces to the block-layout permutation
(B, C, H, W) -> (B, C, 1, H, W) with out[b, c, 0] being the row-major flatten of
the (H/8, W/8, 8, 8) block view of the input.

Strategy (pure data parallelism over batch, 8 cores, no communication):
  - core k handles 4 batches x 3 channels = 12 images of 512x512 f32 (12 MiB).
  - Input viewed as [768, 4096]: each row chunk = 8 consecutive image rows
    (16 KiB, DRAM-contiguous) -> one SBUF partition.
  - On-chip shuffle per partition (vector engine): free-dim permutation
    (r, bw, c) -> (bw, r, c) with r=8 image rows, bw=64 block-cols, c=8.
  - Output [768, 4096] is then DRAM-contiguous per partition as well, so both
    DMAs run at full 16 KiB/partition descriptor efficiency.
"""

import numpy as np

_B, _C, _H, _W = 32, 3, 512, 512
_N_CORES = 8
_ROWS_PER_CORE = (_B // _N_CORES) * _C * (_H // 8)  # 768 chunks per core
_COLS = 8 * _W                                      # 4096 f32 per chunk
_N_TILES = _ROWS_PER_CORE // 128                    # 6 tiles of [128, 4096]

_nc_cache = None


def _build():
    import concourse.mybir as mybir
    from concourse import bacc
    from concourse.tile import TileContext

    nc = bacc.Bacc(
        "TRN2", target_bir_lowering=False, debug=False, num_devices=_N_CORES
    )
    x = nc.dram_tensor(
        "x", (_ROWS_PER_CORE, _COLS), mybir.dt.float32, kind="ExternalInput"
    ).ap()
    y = nc.dram_tensor(
        "y", (_ROWS_PER_CORE, _COLS), mybir.dt.float32, kind="ExternalOutput"
    ).ap()

    with TileContext(nc) as tc:
        with tc.tile_pool(name="io", bufs=3) as pool:
            for t in range(_N_TILES):
                tin = pool.tile([128, _COLS], mybir.dt.float32, tag="in")
                nc.sync.dma_start(out=tin[:, :], in_=x[t * 128:(t + 1) * 128, :])
                tout = pool.tile([128, _COLS], mybir.dt.float32, tag="out")
                src = tin[:, :].rearrange("p (r bw c) -> p bw r c", r=8, bw=64, c=8)
                dst = tout[:, :].rearrange("p (bw r c) -> p bw r c", bw=64, r=8, c=8)
                nc.vector.tensor_copy(out=dst, in_=src)
                nc.sync.dma_start(out=y[t * 128:(t + 1) * 128, :], in_=tout[:, :])
    nc.compile()
    return nc


def kernel(x: np.ndarray) -> np.ndarray:
    from concourse import bass_utils

    global _nc_cache
    if _nc_cache is None:
        _nc_cache = _build()
    nc = _nc_cache

    x = np.ascontiguousarray(x, dtype=np.float32)
    xs = x.reshape(_N_CORES, _ROWS_PER_CORE, _COLS)
    in_maps = [{"x": xs[k]} for k in range(_N_CORES)]
    res = bass_utils.run_bass_kernel_spmd(
        nc, in_maps, core_ids=list(range(_N_CORES))
    )
    ys = np.stack([res.results[k]["y"] for k in range(_N_CORES)], axis=0)
    return ys.reshape(_B, _C, 1, _H, _W)
